# revision 1
# baseline (speedup 1.0000x reference)
"""AntModel forward on 8 TRN2 NeuronCores (Bass/Tile, two-NEFF SPMD).

Math: the reference is three scatter-add layers with routing tables
dest_i = argmax(W_i, axis=1) and relu between layers. Counts are
non-negative, so the relus are no-ops and the routing composes:
out = x @ P1 @ P2 @ P3 = scatter of x by r = dest3[dest2[dest1]].

Distribution (8 cores, K-sharding over the 4096 source rows):
  NEFF A (memory-bound part): core c scans rows [512c, 512c+512) of
  W1/W2/W3 (18 MB/core) and emits the per-shard argmax tables [3*512]
  (one full-row DVE max + max_index per 128-row tile; the [128, T]
  per-partition results are PE-transposed so the DRAM write is
  contiguous).
  Host: concatenates the shard tables and composes r = d3[d2[d1]]
  (4096 integer lookups), then reshards r.
  NEFF B: core c builds one-hot(r) tiles via iota + is_equal and
  accumulates x[:, shard] @ onehot over its 512 sources on the
  TensorEngine (bf16 operands, f32 PSUM — exact for integer counts),
  emitting a partial [256, 1024]. Host sums the 8 partials (the
  K-shard unshard step).
"""

import numpy as np

import concourse.bacc as bacc
import concourse.tile as tile
import concourse.mybir as mybir
from concourse import masks
from concourse import bass_utils

N_CORES = 8
B = 256
S = 4096
SH = S // N_CORES  # 512 rows per core
N1, N2, N3 = 4096, 4096, 1024
P = 128
T = SH // P  # 4 tiles of 128 rows per shard
F32 = mybir.dt.float32
BF16 = mybir.dt.bfloat16
U32 = mybir.dt.uint32
I32 = mybir.dt.int32

_CACHE = {}


def _build_a():
    nc = bacc.Bacc("TRN2", target_bir_lowering=False, debug=False, num_devices=N_CORES)
    W1s = nc.dram_tensor("W1s", [SH, N1], F32, kind="ExternalInput")
    W2s = nc.dram_tensor("W2s", [SH, N2], F32, kind="ExternalInput")
    W3s = nc.dram_tensor("W3s", [SH, N3], F32, kind="ExternalInput")
    dA = nc.dram_tensor("dA", [3 * SH], F32, kind="ExternalOutput")

    with tile.TileContext(nc) as tc:
        with (
            tc.tile_pool(name="w", bufs=3) as wpool,
            tc.tile_pool(name="sm", bufs=1) as sm,
            tc.tile_pool(name="psT", bufs=1, space="PSUM") as psT,
        ):
            ident = sm.tile([P, P], F32, tag="ident")
            masks.make_identity(nc, ident[:])

            d123f = sm.tile([P, 3 * T], F32, tag="d123f")
            # W3 first so its small (0.5 MB) tiles land quickly and the
            # vector engine starts early; 2 MB tiles are loaded as two
            # half-DMAs so consecutive transfers overlap across queues.
            for wi, wt, n_out in ((2, W3s, N3), (0, W1s, N1), (1, W2s, N2)):
                ix = sm.tile([P, 8 * T], U32, tag=f"ix{wi}", name=f"ix{wi}")
                for t in range(T):
                    w = wpool.tile([P, n_out], F32, tag="wtile")
                    half = n_out // 2
                    nc.sync.dma_start(w[:, 0:half], wt[P * t : P * (t + 1), 0:half])
                    nc.sync.dma_start(
                        w[:, half:n_out], wt[P * t : P * (t + 1), half:n_out]
                    )
                    mx = sm.tile([P, 8], F32, tag=f"mx_{wi}_{t}")
                    nc.vector.max(mx[:], w[:])
                    nc.vector.max_index(ix[:, 8 * t : 8 * t + 8], mx[:], w[:])
                nc.vector.tensor_copy(
                    d123f[:, T * wi : T * (wi + 1)], ix[:, 0 : 8 * T : 8]
                )

            dT_ps = psT.tile([3 * T, P], F32, tag="dT_ps")
            nc.tensor.transpose(dT_ps[:], d123f[:], ident[:])
            dT = sm.tile([3 * T, P], F32, tag="dT")
            nc.scalar.copy(dT[:], dT_ps[:])
            nc.scalar.dma_start(dA.rearrange("(r p) -> r p", p=P), dT[:])

    nc.compile()
    return nc


def _build_b():
    nc = bacc.Bacc("TRN2", target_bir_lowering=False, debug=False, num_devices=N_CORES)
    xT = nc.dram_tensor("xT", [SH, B], F32, kind="ExternalInput")
    rpt = nc.dram_tensor("rpt", [P, T], F32, kind="ExternalInput")
    out = nc.dram_tensor("out", [B, N3], F32, kind="ExternalOutput")

    with tile.TileContext(nc) as tc:
        with (
            tc.tile_pool(name="sm", bufs=1) as sm,
            tc.tile_pool(name="psum", bufs=1, space="PSUM") as psum,
        ):
            iota_t = sm.tile([P, N3], I32, tag="iota")
            nc.gpsimd.iota(iota_t[:], pattern=[[1, N3]], base=0, channel_multiplier=0)
            rp = sm.tile([P, T], F32, tag="rp")
            nc.scalar.dma_start(rp[:], rpt[:, :])
            xbs, ohs = [], []
            for t in range(T):
                xf = sm.tile([P, B], F32, tag=f"xf{t}")
                nc.scalar.dma_start(xf[:], xT[P * t : P * (t + 1), :])
                xb = sm.tile([P, B], BF16, tag=f"xb{t}")
                nc.vector.tensor_copy(xb[:], xf[:])
                xbs.append(xb)
                oh = sm.tile([P, N3], BF16, tag=f"oh{t}")
                nc.vector.tensor_scalar(
                    oh[:], iota_t[:], rp[:, t : t + 1], None, mybir.AluOpType.is_equal
                )
                ohs.append(oh)
            for m in range(B // P):
                for n in range(N3 // 512):
                    pm = psum.tile([P, 512], F32, tag=f"pm{m}{n}", name=f"pm{m}{n}")
                    for t in range(T):
                        nc.tensor.matmul(
                            pm[:],
                            xbs[t][:, P * m : P * (m + 1)],
                            ohs[t][:, 512 * n : 512 * (n + 1)],
                            start=(t == 0),
                            stop=(t == T - 1),
                        )
                    ob = sm.tile([P, 512], F32, tag=f"ob{m}{n}")
                    nc.scalar.copy(ob[:], pm[:])
                    nc.sync.dma_start(
                        out[P * m : P * (m + 1), 512 * n : 512 * (n + 1)], ob[:]
                    )

    nc.compile()
    return nc


def _get_kernels():
    if "a" not in _CACHE:
        _CACHE["a"] = _build_a()
        _CACHE["b"] = _build_b()
    return _CACHE["a"], _CACHE["b"]


def run_neffs(x, W1, W2, W3, trace=False, tmpdir_a=None, tmpdir_b=None):
    """Run both NEFFs; returns (out_full, exec_a_ns, exec_b_ns)."""
    nc_a, nc_b = _get_kernels()

    maps_a = []
    for c in range(N_CORES):
        sl = slice(SH * c, SH * (c + 1))
        maps_a.append(
            {
                "W1s": np.ascontiguousarray(W1[sl, :], dtype=np.float32),
                "W2s": np.ascontiguousarray(W2[sl, :], dtype=np.float32),
                "W3s": np.ascontiguousarray(W3[sl, :], dtype=np.float32),
            }
        )
    res_a = bass_utils.run_bass_kernel_spmd(
        nc_a, maps_a, core_ids=list(range(N_CORES)), trace=trace, tmpdir=tmpdir_a
    )

    # host: concatenate shard tables, compose routes r = d3[d2[d1]]
    tabs = [np.ravel(r["dA"]) for r in res_a.results]
    d1 = np.concatenate([t[0:SH] for t in tabs]).astype(np.int64)
    d2 = np.concatenate([t[SH : 2 * SH] for t in tabs]).astype(np.int64)
    d3 = np.concatenate([t[2 * SH : 3 * SH] for t in tabs]).astype(np.int64)
    r_full = d3[d2[d1]]  # [4096] values in [0, 1024)

    maps_b = []
    for c in range(N_CORES):
        sl = slice(SH * c, SH * (c + 1))
        r_c = r_full[sl].astype(np.float32)
        maps_b.append(
            {
                "xT": np.ascontiguousarray(x[:, sl].T.astype(np.float32)),
                "rpt": np.ascontiguousarray(r_c.reshape(T, P).T),
            }
        )
    res_b = bass_utils.run_bass_kernel_spmd(
        nc_b, maps_b, core_ids=list(range(N_CORES)), trace=trace, tmpdir=tmpdir_b
    )

    out = np.sum(
        [r["out"] for r in res_b.results], axis=0, dtype=np.float64
    ).astype(np.float32)
    exec_a = res_a.exec_time_ns
    exec_b = res_b.exec_time_ns
    return out, exec_a, exec_b


def kernel(x, W1, W2, W3):
    x = np.asarray(x)
    W1 = np.asarray(W1, dtype=np.float32)
    W2 = np.asarray(W2, dtype=np.float32)
    W3 = np.asarray(W3, dtype=np.float32)
    out, _, _ = run_neffs(x, W1, W2, W3, trace=False)
    return out


# revision 2
# speedup vs baseline: 1.3271x; 1.3271x over previous
"""AntModel forward on 8 TRN2 NeuronCores (Bass/Tile, two-NEFF SPMD).

Math: the reference is three scatter-add layers with routing tables
dest_i = argmax(W_i, axis=1) and relu between layers. Counts are
non-negative, so the relus are no-ops and the routing composes:
out = x @ P1 @ P2 @ P3 = scatter of x by r = dest3[dest2[dest1]].

Distribution (8 cores, K-sharding over the 4096 source rows):
  NEFF A (memory-bound part): core c scans rows [512c, 512c+512) of
  W1/W2/W3 (18 MB/core) and emits the per-shard argmax tables [3*512]
  (one full-row DVE max + max_index per 128-row tile; the [128, T]
  per-partition results are PE-transposed so the DRAM write is
  contiguous).
  Host: concatenates the shard tables and composes r = d3[d2[d1]]
  (4096 integer lookups), then reshards r.
  NEFF B: core c builds one-hot(r) tiles via iota + is_equal and
  accumulates x[:, shard] @ onehot over its 512 sources on the
  TensorEngine (bf16 operands, f32 PSUM — exact for integer counts),
  emitting a partial [256, 1024]. Host sums the 8 partials (the
  K-shard unshard step).
"""

import numpy as np

import concourse.bacc as bacc
import concourse.tile as tile
import concourse.mybir as mybir
from concourse import masks
from concourse import bass_utils

N_CORES = 8
B = 256
S = 4096
SH = S // N_CORES  # 512 rows per core
N1, N2, N3 = 4096, 4096, 1024
P = 128
T = SH // P  # 4 tiles of 128 rows per shard
F32 = mybir.dt.float32
BF16 = mybir.dt.bfloat16
U32 = mybir.dt.uint32
I32 = mybir.dt.int32

_CACHE = {}


def _build_a():
    nc = bacc.Bacc("TRN2", target_bir_lowering=False, debug=False, num_devices=N_CORES)
    W1s = nc.dram_tensor("W1s", [SH, N1], F32, kind="ExternalInput")
    W2s = nc.dram_tensor("W2s", [SH, N2], F32, kind="ExternalInput")
    W3s = nc.dram_tensor("W3s", [SH, N3], F32, kind="ExternalInput")
    dA = nc.dram_tensor("dA", [3 * SH], F32, kind="ExternalOutput")

    with tile.TileContext(nc) as tc:
        with (
            tc.tile_pool(name="w", bufs=3) as wpool,
            tc.tile_pool(name="sm", bufs=1) as sm,
            tc.tile_pool(name="psT", bufs=1, space="PSUM") as psT,
        ):
            ident = sm.tile([P, P], F32, tag="ident")
            masks.make_identity(nc, ident[:])

            # W3 first so its small (0.5 MB) tiles land quickly and the
            # vector engine starts early; 2 MB tiles are loaded as split
            # DMAs so consecutive transfers overlap across queues. Each W's
            # dest column block is transposed and written out as soon as its
            # group finishes, so only the last group's chain is on the tail.
            dAv = dA.rearrange("(g r p) -> g r p", g=3, p=P)
            for wi, wt, n_out in ((2, W3s, N3), (0, W1s, N1), (1, W2s, N2)):
                ix = sm.tile([P, 8 * T], U32, tag=f"ix{wi}", name=f"ix{wi}")
                for t in range(T):
                    w = wpool.tile([P, n_out], F32, tag="wtile")
                    nsp = 4 if (wi, t) == (2, 0) else 2
                    q = n_out // nsp
                    for qq in range(nsp):
                        nc.sync.dma_start(
                            w[:, q * qq : q * (qq + 1)],
                            wt[P * t : P * (t + 1), q * qq : q * (qq + 1)],
                        )
                    mx = sm.tile([P, 8], F32, tag=f"mx_{wi}_{t}")
                    nc.vector.max(mx[:], w[:])
                    nc.vector.max_index(ix[:, 8 * t : 8 * t + 8], mx[:], w[:])
                dwf = sm.tile([P, T], F32, tag=f"dwf{wi}", name=f"dwf{wi}")
                nc.vector.tensor_copy(dwf[:], ix[:, 0 : 8 * T : 8])
                dT_ps = psT.tile([T, P], F32, tag="dT_ps", name=f"dT_ps{wi}")
                nc.tensor.transpose(dT_ps[:], dwf[:], ident[:])
                dT = sm.tile([T, P], F32, tag=f"dT{wi}", name=f"dT{wi}")
                nc.scalar.copy(dT[:], dT_ps[:])
                nc.scalar.dma_start(dAv[wi], dT[:])

    nc.compile()
    return nc


def _build_b():
    nc = bacc.Bacc("TRN2", target_bir_lowering=False, debug=False, num_devices=N_CORES)
    xT = nc.dram_tensor("xT", [SH, B], F32, kind="ExternalInput")
    rpt = nc.dram_tensor("rpt", [P, T], F32, kind="ExternalInput")
    out = nc.dram_tensor("out", [B, N3], F32, kind="ExternalOutput")

    with tile.TileContext(nc) as tc:
        with (
            tc.tile_pool(name="sm", bufs=1) as sm,
            tc.tile_pool(name="psum", bufs=1, space="PSUM") as psum,
        ):
            iota_t = sm.tile([P, N3], I32, tag="iota")
            nc.gpsimd.iota(iota_t[:], pattern=[[1, N3]], base=0, channel_multiplier=0)
            rp = sm.tile([P, T], F32, tag="rp")
            nc.scalar.dma_start(rp[:], rpt[:, :])
            xbs, ohs = [], []
            for t in range(T):
                xf = sm.tile([P, B], F32, tag=f"xf{t}")
                nc.scalar.dma_start(xf[:], xT[P * t : P * (t + 1), :])
                xb = sm.tile([P, B], BF16, tag=f"xb{t}")
                nc.vector.tensor_copy(xb[:], xf[:])
                xbs.append(xb)
                oh = sm.tile([P, N3], BF16, tag=f"oh{t}")
                nc.vector.tensor_scalar(
                    oh[:], iota_t[:], rp[:, t : t + 1], None, mybir.AluOpType.is_equal
                )
                ohs.append(oh)
            for m in range(B // P):
                for n in range(N3 // 512):
                    pm = psum.tile([P, 512], F32, tag=f"pm{m}{n}", name=f"pm{m}{n}")
                    for t in range(T):
                        nc.tensor.matmul(
                            pm[:],
                            xbs[t][:, P * m : P * (m + 1)],
                            ohs[t][:, 512 * n : 512 * (n + 1)],
                            start=(t == 0),
                            stop=(t == T - 1),
                        )
                    ob = sm.tile([P, 512], F32, tag=f"ob{m}{n}")
                    nc.scalar.copy(ob[:], pm[:])
                    nc.sync.dma_start(
                        out[P * m : P * (m + 1), 512 * n : 512 * (n + 1)], ob[:]
                    )

    nc.compile()
    return nc


def _get_kernels():
    if "a" not in _CACHE:
        _CACHE["a"] = _build_a()
        _CACHE["b"] = _build_b()
    return _CACHE["a"], _CACHE["b"]


def run_neffs(x, W1, W2, W3, trace=False, tmpdir_a=None, tmpdir_b=None):
    """Run both NEFFs; returns (out_full, exec_a_ns, exec_b_ns)."""
    nc_a, nc_b = _get_kernels()

    maps_a = []
    for c in range(N_CORES):
        sl = slice(SH * c, SH * (c + 1))
        maps_a.append(
            {
                "W1s": np.ascontiguousarray(W1[sl, :], dtype=np.float32),
                "W2s": np.ascontiguousarray(W2[sl, :], dtype=np.float32),
                "W3s": np.ascontiguousarray(W3[sl, :], dtype=np.float32),
            }
        )
    res_a = bass_utils.run_bass_kernel_spmd(
        nc_a, maps_a, core_ids=list(range(N_CORES)), trace=trace, tmpdir=tmpdir_a
    )

    # host: concatenate shard tables, compose routes r = d3[d2[d1]]
    tabs = [np.ravel(r["dA"]) for r in res_a.results]
    d1 = np.concatenate([t[0:SH] for t in tabs]).astype(np.int64)
    d2 = np.concatenate([t[SH : 2 * SH] for t in tabs]).astype(np.int64)
    d3 = np.concatenate([t[2 * SH : 3 * SH] for t in tabs]).astype(np.int64)
    r_full = d3[d2[d1]]  # [4096] values in [0, 1024)

    maps_b = []
    for c in range(N_CORES):
        sl = slice(SH * c, SH * (c + 1))
        r_c = r_full[sl].astype(np.float32)
        maps_b.append(
            {
                "xT": np.ascontiguousarray(x[:, sl].T.astype(np.float32)),
                "rpt": np.ascontiguousarray(r_c.reshape(T, P).T),
            }
        )
    res_b = bass_utils.run_bass_kernel_spmd(
        nc_b, maps_b, core_ids=list(range(N_CORES)), trace=trace, tmpdir=tmpdir_b
    )

    out = np.sum(
        [r["out"] for r in res_b.results], axis=0, dtype=np.float64
    ).astype(np.float32)
    exec_a = res_a.exec_time_ns
    exec_b = res_b.exec_time_ns
    return out, exec_a, exec_b


def kernel(x, W1, W2, W3):
    x = np.asarray(x)
    W1 = np.asarray(W1, dtype=np.float32)
    W2 = np.asarray(W2, dtype=np.float32)
    W3 = np.asarray(W3, dtype=np.float32)
    out, _, _ = run_neffs(x, W1, W2, W3, trace=False)
    return out


# revision 3
# speedup vs baseline: 1.5538x; 1.1708x over previous
"""AntModel forward on 8 TRN2 NeuronCores (Bass/Tile, two-NEFF SPMD).

Math: the reference is three scatter-add layers with routing tables
dest_i = argmax(W_i, axis=1) and relu between layers. Counts are
non-negative, so the relus are no-ops and the routing composes:
out = x @ P1 @ P2 @ P3 = scatter of x by r = dest3[dest2[dest1]].

Distribution (8 cores, K-sharding over the 4096 source rows):
  NEFF A (memory-bound part): core c scans rows [512c, 512c+512) of
  W1/W2/W3 (18 MB/core) and emits the per-shard argmax tables [3*512]
  (one full-row DVE max + max_index per 128-row tile; the [128, T]
  per-partition results are PE-transposed so the DRAM write is
  contiguous).
  Host: concatenates the shard tables and composes r = d3[d2[d1]]
  (4096 integer lookups), then reshards r.
  NEFF B: core c builds one-hot(r) tiles via iota + is_equal and
  accumulates x[:, shard] @ onehot over its 512 sources on the
  TensorEngine (bf16 operands, f32 PSUM — exact for integer counts),
  emitting a partial [256, 1024]. Host sums the 8 partials (the
  K-shard unshard step).
"""

import numpy as np

import concourse.bacc as bacc
import concourse.tile as tile
import concourse.mybir as mybir
from concourse import masks
from concourse import bass_utils

N_CORES = 8
B = 256
S = 4096
SH = S // N_CORES  # 512 rows per core
N1, N2, N3 = 4096, 4096, 1024
P = 128
T = SH // P  # 4 tiles of 128 rows per shard
F32 = mybir.dt.float32
BF16 = mybir.dt.bfloat16
U32 = mybir.dt.uint32
I32 = mybir.dt.int32

_CACHE = {}


def _build_a():
    nc = bacc.Bacc("TRN2", target_bir_lowering=False, debug=False, num_devices=N_CORES)
    W1s = nc.dram_tensor("W1s", [SH, N1], F32, kind="ExternalInput")
    W2s = nc.dram_tensor("W2s", [SH, N2], F32, kind="ExternalInput")
    W3s = nc.dram_tensor("W3s", [SH, N3], F32, kind="ExternalInput")
    dA = nc.dram_tensor("dA", [3 * SH], F32, kind="ExternalOutput")

    with tile.TileContext(nc) as tc:
        with (
            tc.tile_pool(name="w", bufs=3) as wpool,
            tc.tile_pool(name="sm", bufs=1) as sm,
            tc.tile_pool(name="psT", bufs=1, space="PSUM") as psT,
        ):
            ident = sm.tile([P, P], F32, tag="ident")
            masks.make_identity(nc, ident[:])

            d123f = sm.tile([P, 3 * T], F32, tag="d123f")
            # W3 first so its small (0.5 MB) tiles land quickly and the
            # vector engine starts early; 2 MB tiles are loaded as two
            # half-DMAs so consecutive transfers overlap across queues.
            for wi, wt, n_out in ((2, W3s, N3), (0, W1s, N1), (1, W2s, N2)):
                ix = sm.tile([P, 8 * T], U32, tag=f"ix{wi}", name=f"ix{wi}")
                for t in range(T):
                    w = wpool.tile([P, n_out], F32, tag="wtile")
                    half = n_out // 2
                    nc.sync.dma_start(w[:, 0:half], wt[P * t : P * (t + 1), 0:half])
                    nc.sync.dma_start(
                        w[:, half:n_out], wt[P * t : P * (t + 1), half:n_out]
                    )
                    mx = sm.tile([P, 8], F32, tag=f"mx_{wi}_{t}")
                    nc.vector.max(mx[:], w[:])
                    nc.vector.max_index(ix[:, 8 * t : 8 * t + 8], mx[:], w[:])
                nc.vector.tensor_copy(
                    d123f[:, T * wi : T * (wi + 1)], ix[:, 0 : 8 * T : 8]
                )

            dT_ps = psT.tile([3 * T, P], F32, tag="dT_ps")
            nc.tensor.transpose(dT_ps[:], d123f[:], ident[:])
            dT = sm.tile([3 * T, P], F32, tag="dT")
            nc.scalar.copy(dT[:], dT_ps[:])
            nc.scalar.dma_start(dA.rearrange("(r p) -> r p", p=P), dT[:])

    nc.compile()
    return nc


def _build_b():
    nc = bacc.Bacc("TRN2", target_bir_lowering=False, debug=False, num_devices=N_CORES)
    xT = nc.dram_tensor("xT", [SH, B], F32, kind="ExternalInput")
    rpt = nc.dram_tensor("rpt", [P, T], F32, kind="ExternalInput")
    out = nc.dram_tensor("out", [B, N3], F32, kind="ExternalOutput")

    with tile.TileContext(nc) as tc:
        with (
            tc.tile_pool(name="sm", bufs=1) as sm,
            tc.tile_pool(name="psum", bufs=1, space="PSUM") as psum,
        ):
            iota_t = sm.tile([P, N3], I32, tag="iota")
            nc.gpsimd.iota(iota_t[:], pattern=[[1, N3]], base=0, channel_multiplier=0)
            rp = sm.tile([P, T], F32, tag="rp")
            nc.scalar.dma_start(rp[:], rpt[:, :])
            xbs, ohs = [], []
            for t in range(T):
                xf = sm.tile([P, B], F32, tag=f"xf{t}")
                nc.scalar.dma_start(xf[:], xT[P * t : P * (t + 1), :])
                xb = sm.tile([P, B], BF16, tag=f"xb{t}")
                nc.vector.tensor_copy(xb[:], xf[:])
                xbs.append(xb)
                oh = sm.tile([P, N3], BF16, tag=f"oh{t}")
                nc.vector.tensor_scalar(
                    oh[:], iota_t[:], rp[:, t : t + 1], None, mybir.AluOpType.is_equal
                )
                ohs.append(oh)
            for m in range(B // P):
                for n in range(N3 // 512):
                    pm = psum.tile([P, 512], F32, tag=f"pm{m}{n}", name=f"pm{m}{n}")
                    for t in range(T):
                        nc.tensor.matmul(
                            pm[:],
                            xbs[t][:, P * m : P * (m + 1)],
                            ohs[t][:, 512 * n : 512 * (n + 1)],
                            start=(t == 0),
                            stop=(t == T - 1),
                        )
                    ob = sm.tile([P, 512], F32, tag=f"ob{m}{n}")
                    nc.scalar.copy(ob[:], pm[:])
                    nc.sync.dma_start(
                        out[P * m : P * (m + 1), 512 * n : 512 * (n + 1)], ob[:]
                    )

    nc.compile()
    return nc


def _get_kernels():
    if "a" not in _CACHE:
        _CACHE["a"] = _build_a()
        _CACHE["b"] = _build_b()
    return _CACHE["a"], _CACHE["b"]


def run_neffs(x, W1, W2, W3, trace=False, tmpdir_a=None, tmpdir_b=None):
    """Run both NEFFs; returns (out_full, exec_a_ns, exec_b_ns)."""
    nc_a, nc_b = _get_kernels()

    maps_a = []
    for c in range(N_CORES):
        sl = slice(SH * c, SH * (c + 1))
        maps_a.append(
            {
                "W1s": np.ascontiguousarray(W1[sl, :], dtype=np.float32),
                "W2s": np.ascontiguousarray(W2[sl, :], dtype=np.float32),
                "W3s": np.ascontiguousarray(W3[sl, :], dtype=np.float32),
            }
        )
    res_a = bass_utils.run_bass_kernel_spmd(
        nc_a, maps_a, core_ids=list(range(N_CORES)), trace=trace, tmpdir=tmpdir_a
    )

    # host: concatenate shard tables, compose routes r = d3[d2[d1]]
    tabs = [np.ravel(r["dA"]) for r in res_a.results]
    d1 = np.concatenate([t[0:SH] for t in tabs]).astype(np.int64)
    d2 = np.concatenate([t[SH : 2 * SH] for t in tabs]).astype(np.int64)
    d3 = np.concatenate([t[2 * SH : 3 * SH] for t in tabs]).astype(np.int64)
    r_full = d3[d2[d1]]  # [4096] values in [0, 1024)

    maps_b = []
    for c in range(N_CORES):
        sl = slice(SH * c, SH * (c + 1))
        r_c = r_full[sl].astype(np.float32)
        maps_b.append(
            {
                "xT": np.ascontiguousarray(x[:, sl].T.astype(np.float32)),
                "rpt": np.ascontiguousarray(r_c.reshape(T, P).T),
            }
        )
    res_b = bass_utils.run_bass_kernel_spmd(
        nc_b, maps_b, core_ids=list(range(N_CORES)), trace=trace, tmpdir=tmpdir_b
    )

    out = np.sum(
        [r["out"] for r in res_b.results], axis=0, dtype=np.float64
    ).astype(np.float32)
    exec_a = res_a.exec_time_ns
    exec_b = res_b.exec_time_ns
    return out, exec_a, exec_b


def kernel(x, W1, W2, W3):
    x = np.asarray(x)
    W1 = np.asarray(W1, dtype=np.float32)
    W2 = np.asarray(W2, dtype=np.float32)
    W3 = np.asarray(W3, dtype=np.float32)
    out, _, _ = run_neffs(x, W1, W2, W3, trace=False)
    return out
